# revision 29
# baseline (speedup 1.0000x reference)
"""CGNN message-passing kernel for 8 trn2 NeuronCores.

Algorithm (per image (b,a), image = [S=768, T=14] grid):
  x = pw_vh(dw_hh(concat(h2,h1))) + pw_vp(dw_hp(pe)) + beta   (conv1 + pe branch)
  x = relu(x)
  y = pw_ov(dw_oh(x)) + beta2                                 (conv2)

Layout strategy: channel-major SBUF tiles [(chan,t) partitions, s free].
The DRAM<->channel-major layout conversion is done HOST-SIDE in numpy inside
kernel() (pure transpose + bf16 cast, outside HW exec), so images DMA
straight into [84, S] tiles and results DMA straight out of [28, S] tiles —
no PE transposes, no ingest/egress shuffle copies, and half the input DMA
bytes (bf16).

Depthwise 3x3 convs become accumulating matmuls with host-precomputed banded
lhsT matrices encoding the t-direction taps (T=14 blocks on the partition
axis). On-chip compute runs in bf16 (PSUM accumulation stays fp32): bf16
matmuls stream 1 cycle/row vs fp32's 4.

conv1: dw = 3 accumulating matmuls (one per s-shift ds) with out zero-padded
to 96 rows; pw streams a combined [124, s] rhs = [dw-output (96 rows, 84
live); pe-branch dw output (28 rows at base 96)] so the pe contribution rides
in the same matmul. conv1's bias is applied in the relu stage (activation
bias / scalar_tensor_tensor). conv2: dw+pw fused into banded lhsT [m, 96]
whose out dim packs the 3 s-taps in 32-row blocks; tap alignment is resolved
after the matmul by two shifted adds over a full-row [96, 770] PSUM tile.
All engine reads/writes keep quadrant-aligned (32x) partition bases.

The per-image work is emitted as a 4-stage software pipeline (ingest-DMA,
dw, pw, conv2+out-DMA), each stage one image behind the previous, so every
PE group's dependencies are a full iteration old: the PE runs gap-free at
max p-state.

Sharding: data-parallel over batch B=16 -> 2 batches per core.
"""

import numpy as np
import ml_dtypes
from contextlib import ExitStack

import concourse.bass as bass
import concourse.bacc as bacc
import concourse.tile as tile
from concourse import mybir
from concourse.bass_utils import run_bass_kernel_spmd

F32 = mybir.dt.float32
BF16 = mybir.dt.bfloat16
NPBF16 = ml_dtypes.bfloat16
B, S, T, A = 16, 768, 14, 16
HK0, PEK0, U, K1 = 6, 2, 32, 2
NCORES = 8
BPC = B // NCORES          # batches per core
NIMG = BPC * A             # images (antenna-planes) per core
SP = S + 2                 # s-padded width (zero col at 0 and S+1)
UCH = [9, 9, 9, 5]         # u-chunk sizes (32 = 9+9+9+5)
UOF = [0, 9, 18, 27]
SCH = [(0, 384), (384, 384)]   # s chunks (PSUM bank = 512 fp32 max)
ZCH = [(0, 512), (512, 258)]   # conv2 chunks inside one [96, 770] psum tile


def _tband(w_t, n_t=T):
    """[n_t, n_t] band matrix M[t, t'] = w_t[t - t' + 1] (3-tap, SAME pad)."""
    m = np.zeros((n_t, n_t), np.float32)
    for t in range(n_t):
        for tp in range(n_t):
            dt = t - tp + 1
            if 0 <= dt <= 2:
                m[t, tp] = w_t[dt]
    return m


def build_consts(w_hh, b_hh, w_vh, b_vh, w_hp, b_hp, w_vp, b_vp,
                 w_oh, b_oh, w_ov, b_ov):
    """Host-side precompute of all lhsT matrices. Returns dict name->array."""
    w_hh = w_hh[:, :, 0, :]   # [3,3,6]
    w_hp = w_hp[:, :, 0, :]   # [3,3,2]
    w_oh = w_oh[:, :, 0, :]   # [3,3,32]

    # conv1 depthwise band: [3, 84, 96] (12 zero out-cols pad to 96 rows of
    # PSUM so the dq->hdc copy is one 96-row base-0 write)
    B1 = np.zeros((3, 6 * T, 96), np.float32)
    for ds in range(3):
        for g in range(6):
            B1[ds, g * T:(g + 1) * T, g * T:(g + 1) * T] = _tband(w_hh[ds, :, g])

    # conv1 pointwise, combined contraction rows:
    #   rows 0..83: (g,t) h-dw output, 84..95: zero pad, 96..123: (c,t) pe-dw
    W1 = np.zeros((124, sum(u * T for u in UCH)), np.float32)
    col = 0
    for uc in range(4):
        for ul in range(UCH[uc]):
            u = UOF[uc] + ul
            for g in range(6):
                W1[g * T:(g + 1) * T, col:col + T] = np.eye(T, dtype=np.float32) * w_vh[g, u]
            for c in range(2):
                W1[96 + c * T:96 + (c + 1) * T, col:col + T] = \
                    np.eye(T, dtype=np.float32) * w_vp[c, u]
            col += T

    # conv1 total bias, applied at the relu stage: [126, 4], col uc, row (ul,t)
    beta = (b_vh + w_vh.T @ b_hh + b_vp + w_vp.T @ b_hp).astype(np.float32)  # [32]
    BV = np.zeros((126, 4), np.float32)
    for uc in range(4):
        for ul in range(UCH[uc]):
            BV[ul * T:(ul + 1) * T, uc] = beta[UOF[uc] + ul]

    # pe depthwise band: [3, 28, 28]
    Bpe = np.zeros((3, 2 * T, 2 * T), np.float32)
    for ds in range(3):
        for c in range(2):
            Bpe[ds, c * T:(c + 1) * T, c * T:(c + 1) * T] = _tband(w_hp[ds, :, c])

    # conv2 fused band (dw_oh folded with pw_ov), s-taps packed in out dim as
    # 32-row blocks: col block uc: [uch*14, 96]; rows (u_local,t),
    # cols (ds*32 + k*14 + t'), rows ds*32+28..ds*32+31 zero
    B2 = np.zeros((9 * T, 4 * 96), np.float32)
    for uc in range(4):
        for ul in range(UCH[uc]):
            u = UOF[uc] + ul
            for ds in range(3):
                band = _tband(w_oh[ds, :, u])          # [T, T]
                for k in range(K1):
                    r0, c0 = ul * T, uc * 96 + ds * 32 + k * T
                    B2[r0:r0 + T, c0:c0 + T] = band * w_ov[u, k]

    beta2 = (b_ov + w_ov.T @ b_oh).astype(np.float32)  # [2]
    b2vec = np.repeat(beta2, T).reshape(K1 * T, 1).astype(np.float32)  # rows (k,t)

    bf = lambda x: np.ascontiguousarray(x.astype(NPBF16))
    return {
        "wB1": bf(B1), "wW1": bf(W1), "wBpe": bf(Bpe), "wB2": bf(B2),
        "wb2": np.ascontiguousarray(b2vec),
        "wbv": np.ascontiguousarray(BV),
    }


def _trace_kernel(nc):
    ht_in = nc.dram_tensor("ht_in", [BPC, A, 84, S], BF16, kind="ExternalInput").ap()
    pet_in = nc.dram_tensor("pet_in", [BPC, 28, S], BF16, kind="ExternalInput").ap()
    wB1 = nc.dram_tensor("wB1", [3, 84, 96], BF16, kind="ExternalInput").ap()
    wW1 = nc.dram_tensor("wW1", [124, 448], BF16, kind="ExternalInput").ap()
    wBpe = nc.dram_tensor("wBpe", [3, 28, 28], BF16, kind="ExternalInput").ap()
    wB2 = nc.dram_tensor("wB2", [126, 384], BF16, kind="ExternalInput").ap()
    wb2 = nc.dram_tensor("wb2", [28, 1], F32, kind="ExternalInput").ap()
    wbv = nc.dram_tensor("wbv", [126, 4], F32, kind="ExternalInput").ap()
    y_t = nc.dram_tensor("y_t", [BPC, A, 28, S], F32, kind="ExternalOutput").ap()

    RELU = mybir.ActivationFunctionType.Relu
    IDENT = mybir.ActivationFunctionType.Identity
    ADD = mybir.AluOpType.add
    MAX = mybir.AluOpType.max

    with tile.TileContext(nc) as tc, ExitStack() as ctx:
        wp = ctx.enter_context(tc.tile_pool(name="w", bufs=1))
        ht_p = ctx.enter_context(tc.tile_pool(name="ht", bufs=2))
        hdc_p = ctx.enter_context(tc.tile_pool(name="hdc", bufs=2))
        pedw_p = ctx.enter_context(tc.tile_pool(name="pedw", bufs=2))
        x_p = ctx.enter_context(tc.tile_pool(name="xs", bufs=2))
        ysb_p = ctx.enter_context(tc.tile_pool(name="ysb", bufs=2))
        t1_p = ctx.enter_context(tc.tile_pool(name="t1", bufs=2))
        pdw = ctx.enter_context(tc.tile_pool(name="pdw", bufs=2, space="PSUM"))
        px = ctx.enter_context(tc.tile_pool(name="px", bufs=2, space="PSUM"))
        pz = ctx.enter_context(tc.tile_pool(name="pz", bufs=1, space="PSUM"))

        b1 = wp.tile([84, 3, 96], BF16)
        nc.sync.dma_start(b1[:], wB1.rearrange("d k m -> k d m"))
        w1 = wp.tile([124, 448], BF16)
        nc.sync.dma_start(w1[:], wW1)
        bpe = wp.tile([28, 3, 28], BF16)
        nc.sync.dma_start(bpe[:], wBpe.rearrange("d k m -> k d m"))
        b2t = wp.tile([126, 384], BF16)
        nc.sync.dma_start(b2t[:], wB2)
        b2v = wp.tile([28, 1], F32)
        nc.sync.dma_start(b2v[:], wb2)
        bv = wp.tile([126, 4], F32)
        nc.sync.dma_start(bv[:], wbv)
        zeros = wp.tile([126, S], BF16)
        nc.vector.memset(zeros[:], 0.0)

        peds, peTs = {}, {}
        hTs, hdcs, xss = {}, {}, {}

        def stage_pedma(b):
            peT = ht_p.tile([28, SP], BF16, tag="peT", name="peT")
            nc.vector.memset(peT[:, 0:1], 0.0)
            nc.vector.memset(peT[:, SP - 1:SP], 0.0)
            nc.sync.dma_start(peT[:, 1:1 + S], pet_in[b])
            peTs[b] = peT

        def stage_pecompute(b):
            peT = peTs.pop(b)
            pedw = pedw_p.tile([28, S], BF16, tag="pedw", name="pedw")
            for g, (s0, sn) in enumerate(SCH):
                dq = pdw.tile([96, 384], F32, tag="dq", name="dq")
                for ds in range(3):
                    nc.tensor.matmul(dq[0:28, :sn], bpe[:, ds, :],
                                     peT[:, ds + s0: ds + s0 + sn],
                                     start=(ds == 0), stop=(ds == 2))
                nc.scalar.copy(pedw[:, s0:s0 + sn], dq[0:28, :sn])
            peds[b] = pedw

        def stage_ingest(i):
            b, a = divmod(i, A)
            hT = ht_p.tile([84, SP], BF16, tag="hT", name="hT")
            if i < 2:   # halo cols persist in the tag's 2 rotating buffers
                nc.vector.memset(hT[:, 0:1], 0.0)
                nc.vector.memset(hT[:, SP - 1:SP], 0.0)
            nc.sync.dma_start(hT[:, 1:1 + S], ht_in[b, a])
            hTs[i] = hT

        def stage_dw(i):
            b, a = divmod(i, A)
            hT = hTs.pop(i)
            # combined pw rhs: rows 0..95 h-dw output (84 live), 96..123 pe
            hdc = hdc_p.tile([124, S], BF16, tag="hdc", name="hdc")
            for g, (s0, sn) in enumerate(SCH):
                dq = pdw.tile([96, 384], F32, tag="dq", name="dq")
                for ds in range(3):
                    nc.tensor.matmul(dq[:, :sn], b1[:, ds, :],
                                     hT[:, ds + s0: ds + s0 + sn],
                                     start=(ds == 0), stop=(ds == 2))
                dst = hdc[0:96, s0:s0 + sn]
                if g == 0:
                    nc.vector.tensor_copy(dst, dq[:, :sn])
                else:
                    nc.scalar.copy(dst, dq[:, :sn])
            if a < 2:   # pe rows persist in the tag's 2 rotating buffers
                nc.gpsimd.tensor_copy(hdc[96:124, :], peds[b][:, :])
            hdcs[i] = hdc

        def stage_pw(i):
            hdc = hdcs.pop(i)
            xts = []
            for uc in range(4):
                m = UCH[uc] * T
                c0 = UOF[uc] * T
                xt = x_p.tile([126, SP], BF16, tag=f"x{uc}", name="xt")
                xts.append(xt)
                if i < 2:
                    nc.vector.memset(xt[0:m, 0:1], 0.0)
                    nc.vector.memset(xt[0:m, SP - 1:SP], 0.0)
                xq = px.tile([126, S], F32, tag="xq", name="xq")
                for s0, sn in ((0, 512), (512, 256)):   # one PSUM bank each
                    nc.tensor.matmul(xq[0:m, s0:s0 + sn], w1[:, c0:c0 + m],
                                     hdc[:, s0:s0 + sn], start=True, stop=True)
                dst = xt[0:m, 1:1 + S]
                if uc % 2 == 0:
                    nc.scalar.activation(dst, xq[0:m, :], RELU,
                                         bias=bv[0:m, uc:uc + 1])
                else:
                    nc.vector.scalar_tensor_tensor(
                        dst, xq[0:m, :], bv[0:m, uc:uc + 1],
                        zeros[0:m, :], ADD, MAX)
            xss[i] = xts

        def stage_conv2(i):
            b, a = divmod(i, A)
            xts = xss.pop(i)
            zq = pz.tile([96, 770], F32, tag="zq", name="zq")
            for s0, sn in ZCH:
                for uc in range(4):
                    m = UCH[uc] * T
                    nc.tensor.matmul(zq[:, s0:s0 + sn],
                                     b2t[0:m, uc * 96:(uc + 1) * 96],
                                     xts[uc][0:m, s0:s0 + sn],
                                     start=(uc == 0), stop=(uc == 3))
            s1 = t1_p.tile([28, 768], F32, tag="s1", name="s1")
            nc.scalar.activation(s1[:], zq[0:28, 0:768], IDENT, bias=b2v[:, 0:1])
            t1 = t1_p.tile([28, 768], F32, tag="t1", name="t1")
            nc.vector.tensor_tensor(t1[:], s1[:], zq[32:60, 1:769], ADD)
            yt = ysb_p.tile([28, S], F32, tag="yt", name="yt")
            nc.vector.tensor_tensor(yt[:, :], t1[:], zq[64:92, 2:770], ADD)
            nc.sync.dma_start(y_t[b, a], yt[:])

        stage_pedma(0)
        stage_pecompute(0)
        for i in range(NIMG + 3):
            if i < NIMG:
                b, a = divmod(i, A)
                if a == 1 and b + 1 < BPC:
                    stage_pedma(b + 1)
                if a == 4 and b + 1 < BPC:
                    stage_pecompute(b + 1)
                stage_ingest(i)
            if 0 <= i - 1 < NIMG:
                stage_dw(i - 1)
            if 0 <= i - 2 < NIMG:
                stage_pw(i - 2)
            if 0 <= i - 3 < NIMG:
                stage_conv2(i - 3)
    nc.compile()
    return nc


_CACHED_NC = None


def get_nc():
    global _CACHED_NC
    if _CACHED_NC is None:
        _CACHED_NC = _trace_kernel(
            bacc.Bacc("TRN2", target_bir_lowering=False, debug=False))
    return _CACHED_NC


def make_in_maps(inputs):
    consts = build_consts(
        inputs["w_hh"], inputs["b_hh"], inputs["w_vh"], inputs["b_vh"],
        inputs["w_hp"], inputs["b_hp"], inputs["w_vp"], inputs["b_vp"],
        inputs["w_oh"], inputs["b_oh"], inputs["w_ov"], inputs["b_ov"])
    # host-side layout: channel-major rows (c, t), bf16
    h = np.concatenate([np.asarray(inputs["h2"], np.float32),
                        np.asarray(inputs["h1"], np.float32)], axis=-1)
    h_t = np.ascontiguousarray(
        h.transpose(0, 3, 4, 2, 1).reshape(B, A, 6 * T, S).astype(NPBF16))
    pe_t = np.ascontiguousarray(
        np.asarray(inputs["pe"], np.float32)
        .transpose(0, 3, 2, 1).reshape(B, PEK0 * T, S).astype(NPBF16))
    in_maps = []
    for i in range(NCORES):
        sl = slice(i * BPC, (i + 1) * BPC)
        m = {
            "ht_in": np.ascontiguousarray(h_t[sl]),
            "pet_in": np.ascontiguousarray(pe_t[sl]),
        }
        m.update(consts)
        in_maps.append(m)
    return in_maps


def kernel(**inputs):
    nc = get_nc()
    in_maps = make_in_maps(inputs)
    res = run_bass_kernel_spmd(nc, in_maps, list(range(NCORES)))
    # y_t: [BPC, A, 28(k*T+t), S] -> [BPC, S, T, A, K1]
    out = np.concatenate([r["y_t"] for r in res.results], axis=0)
    out = out.reshape(B, A, K1, T, S).transpose(0, 4, 3, 1, 2)
    return np.ascontiguousarray(out)


# revision 35
# speedup vs baseline: 1.2874x; 1.2874x over previous
"""CGNN message-passing kernel for 8 trn2 NeuronCores.

Algorithm (per image (b,a), image = [S=768, T=14] grid):
  x = pw_vh(dw_hh(concat(h2,h1))) + pw_vp(dw_hp(pe)) + beta   (conv1 + pe branch)
  x = relu(x)
  y = pw_ov(dw_oh(x)) + beta2                                 (conv2)

Layout strategy: channel-major SBUF tiles [(chan,t) partitions, s free].
The DRAM<->channel-major layout conversion is done HOST-SIDE in numpy inside
kernel() (pure transpose + bf16 cast, outside HW exec), so images DMA
straight into [84, S] tiles and results DMA straight out of [28, S] tiles —
no PE transposes, no ingest/egress shuffle copies, and half the input DMA
bytes (bf16).

Depthwise 3x3 convs become accumulating matmuls with host-precomputed banded
lhsT matrices encoding the t-direction taps (T=14 blocks on the partition
axis). On-chip compute runs in bf16 (PSUM accumulation stays fp32): bf16
matmuls stream 1 cycle/row vs fp32's 4.

conv1: dw = 3 accumulating matmuls (one per s-shift ds) with out zero-padded
to 96 rows; pw streams a combined [124, s] rhs = [dw-output (96 rows, 84
live); pe-branch dw output (28 rows at base 96)] so the pe contribution rides
in the same matmul. conv1's bias is applied in the relu stage (activation
bias / scalar_tensor_tensor). conv2: dw+pw fused into banded lhsT [m, 96]
whose out dim packs the 3 s-taps in 32-row blocks; tap alignment is resolved
after the matmul by two shifted adds over a full-row [96, 770] PSUM tile.
All engine reads/writes keep quadrant-aligned (32x) partition bases.

The per-image work is emitted as a 4-stage software pipeline (ingest-DMA,
dw, pw, conv2+out-DMA), each stage one image behind the previous, so every
PE group's dependencies are a full iteration old: the PE runs gap-free at
max p-state.

Sharding: data-parallel over batch B=16 -> 2 batches per core.
"""

import numpy as np
import ml_dtypes
from contextlib import ExitStack

import concourse.bass as bass
import concourse.bacc as bacc
import concourse.tile as tile
from concourse import mybir
from concourse.bass_utils import run_bass_kernel_spmd

F32 = mybir.dt.float32
BF16 = mybir.dt.bfloat16
NPBF16 = ml_dtypes.bfloat16
B, S, T, A = 16, 768, 14, 16
HK0, PEK0, U, K1 = 6, 2, 32, 2
NCORES = 8
BPC = B // NCORES          # batches per core
NIMG = BPC * A             # images (antenna-planes) per core
SP = S + 2                 # s-padded width (zero col at 0 and S+1)
UCH = [9, 9, 9, 5]         # u-chunk sizes (32 = 9+9+9+5)
UOF = [0, 9, 18, 27]
SCH = [(0, 384), (384, 384)]   # s chunks (PSUM bank = 512 fp32 max)
ZCH = [(0, 512), (512, 258)]   # conv2 chunks inside one [96, 770] psum tile


def _tband(w_t, n_t=T):
    """[n_t, n_t] band matrix M[t, t'] = w_t[t - t' + 1] (3-tap, SAME pad)."""
    m = np.zeros((n_t, n_t), np.float32)
    for t in range(n_t):
        for tp in range(n_t):
            dt = t - tp + 1
            if 0 <= dt <= 2:
                m[t, tp] = w_t[dt]
    return m


def build_consts(w_hh, b_hh, w_vh, b_vh, w_hp, b_hp, w_vp, b_vp,
                 w_oh, b_oh, w_ov, b_ov):
    """Host-side precompute of all lhsT matrices. Returns dict name->array."""
    w_hh = w_hh[:, :, 0, :]   # [3,3,6]
    w_hp = w_hp[:, :, 0, :]   # [3,3,2]
    w_oh = w_oh[:, :, 0, :]   # [3,3,32]

    # conv1 depthwise band: [3, 84, 96] (12 zero out-cols pad to 96 rows of
    # PSUM so the dq->hdc copy is one 96-row base-0 write)
    B1 = np.zeros((3, 6 * T, 96), np.float32)
    for ds in range(3):
        for g in range(6):
            B1[ds, g * T:(g + 1) * T, g * T:(g + 1) * T] = _tband(w_hh[ds, :, g])

    # conv1 pointwise, combined contraction rows:
    #   rows 0..83: (g,t) h-dw output, 84..95: zero pad, 96..123: (c,t) pe-dw
    W1 = np.zeros((124, sum(u * T for u in UCH)), np.float32)
    col = 0
    for uc in range(4):
        for ul in range(UCH[uc]):
            u = UOF[uc] + ul
            for g in range(6):
                W1[g * T:(g + 1) * T, col:col + T] = np.eye(T, dtype=np.float32) * w_vh[g, u]
            for c in range(2):
                W1[96 + c * T:96 + (c + 1) * T, col:col + T] = \
                    np.eye(T, dtype=np.float32) * w_vp[c, u]
            col += T

    # conv1 total bias, applied at the relu stage: [126, 4], col uc, row (ul,t)
    beta = (b_vh + w_vh.T @ b_hh + b_vp + w_vp.T @ b_hp).astype(np.float32)  # [32]
    BV = np.zeros((126, 4), np.float32)
    for uc in range(4):
        for ul in range(UCH[uc]):
            BV[ul * T:(ul + 1) * T, uc] = beta[UOF[uc] + ul]

    # pe depthwise band: [3, 28, 28]
    Bpe = np.zeros((3, 2 * T, 2 * T), np.float32)
    for ds in range(3):
        for c in range(2):
            Bpe[ds, c * T:(c + 1) * T, c * T:(c + 1) * T] = _tband(w_hp[ds, :, c])

    # conv2 fused band (dw_oh folded with pw_ov), s-taps packed in out dim as
    # 32-row blocks: col block uc: [uch*14, 96]; rows (u_local,t),
    # cols (ds*32 + k*14 + t'), rows ds*32+28..ds*32+31 zero
    B2 = np.zeros((9 * T, 4 * 96), np.float32)
    for uc in range(4):
        for ul in range(UCH[uc]):
            u = UOF[uc] + ul
            for ds in range(3):
                band = _tband(w_oh[ds, :, u])          # [T, T]
                for k in range(K1):
                    r0, c0 = ul * T, uc * 96 + ds * 32 + k * T
                    B2[r0:r0 + T, c0:c0 + T] = band * w_ov[u, k]

    beta2 = (b_ov + w_ov.T @ b_oh).astype(np.float32)  # [2]
    b2vec = np.repeat(beta2, T).reshape(K1 * T, 1).astype(np.float32)  # rows (k,t)

    bf = lambda x: np.ascontiguousarray(x.astype(NPBF16))
    return {
        "wB1": bf(B1), "wW1": bf(W1), "wBpe": bf(Bpe), "wB2": bf(B2),
        "wb2": np.ascontiguousarray(b2vec),
        "wbv": np.ascontiguousarray(BV),
    }


def _trace_kernel(nc):
    ht_in = nc.dram_tensor("ht_in", [BPC, A, 84, S], BF16, kind="ExternalInput").ap()
    pet_in = nc.dram_tensor("pet_in", [BPC, 28, S], BF16, kind="ExternalInput").ap()
    wB1 = nc.dram_tensor("wB1", [3, 84, 96], BF16, kind="ExternalInput").ap()
    wW1 = nc.dram_tensor("wW1", [124, 448], BF16, kind="ExternalInput").ap()
    wBpe = nc.dram_tensor("wBpe", [3, 28, 28], BF16, kind="ExternalInput").ap()
    wB2 = nc.dram_tensor("wB2", [126, 384], BF16, kind="ExternalInput").ap()
    wb2 = nc.dram_tensor("wb2", [28, 1], F32, kind="ExternalInput").ap()
    wbv = nc.dram_tensor("wbv", [126, 4], F32, kind="ExternalInput").ap()
    y_t = nc.dram_tensor("y_t", [BPC, A, 28, S], F32, kind="ExternalOutput").ap()

    RELU = mybir.ActivationFunctionType.Relu
    IDENT = mybir.ActivationFunctionType.Identity
    ADD = mybir.AluOpType.add
    MAX = mybir.AluOpType.max

    with tile.TileContext(nc) as tc, ExitStack() as ctx:
        wp = ctx.enter_context(tc.tile_pool(name="w", bufs=1))
        ht_p = ctx.enter_context(tc.tile_pool(name="ht", bufs=3))
        hdc_p = ctx.enter_context(tc.tile_pool(name="hdc", bufs=2))
        pedw_p = ctx.enter_context(tc.tile_pool(name="pedw", bufs=2))
        x_p = ctx.enter_context(tc.tile_pool(name="xs", bufs=2))
        ysb_p = ctx.enter_context(tc.tile_pool(name="ysb", bufs=2))
        t1_p = ctx.enter_context(tc.tile_pool(name="t1", bufs=2))
        pdw = ctx.enter_context(tc.tile_pool(name="pdw", bufs=1, space="PSUM"))
        px = ctx.enter_context(tc.tile_pool(name="px", bufs=3, space="PSUM"))
        pz = ctx.enter_context(tc.tile_pool(name="pz", bufs=2, space="PSUM"))

        b1 = wp.tile([84, 3, 96], BF16)
        nc.sync.dma_start(b1[:], wB1.rearrange("d k m -> k d m"))
        w1 = wp.tile([124, 448], BF16)
        nc.sync.dma_start(w1[:], wW1)
        bpe = wp.tile([28, 3, 28], BF16)
        nc.sync.dma_start(bpe[:], wBpe.rearrange("d k m -> k d m"))
        b2t = wp.tile([126, 384], BF16)
        nc.sync.dma_start(b2t[:], wB2)
        b2v = wp.tile([28, 1], F32)
        nc.sync.dma_start(b2v[:], wb2)
        bv = wp.tile([126, 4], F32)
        nc.sync.dma_start(bv[:], wbv)
        zeros = wp.tile([126, 384], BF16)
        nc.vector.memset(zeros[:], 0.0)

        peds, peTs = {}, {}
        hTs, hdcs, xss = {}, {}, {}

        def stage_pedma(b):
            peT = ht_p.tile([28, SP], BF16, tag="peT", name="peT")
            nc.vector.memset(peT[:, 0:1], 0.0)
            nc.vector.memset(peT[:, SP - 1:SP], 0.0)
            nc.sync.dma_start(peT[:, 1:1 + S], pet_in[b])
            peTs[b] = peT

        def stage_pecompute(b):
            peT = peTs.pop(b)
            pedw = pedw_p.tile([28, S], BF16, tag="pedw", name="pedw")
            for g, (s0, sn) in enumerate(SCH):
                dq = pdw.tile([96, 384], F32, tag="dq", name="dq")
                for ds in range(3):
                    nc.tensor.matmul(dq[0:28, :sn], bpe[:, ds, :],
                                     peT[:, ds + s0: ds + s0 + sn],
                                     start=(ds == 0), stop=(ds == 2))
                nc.scalar.copy(pedw[:, s0:s0 + sn], dq[0:28, :sn])
            peds[b] = pedw

        def stage_ingest(i):
            b, a = divmod(i, A)
            hT = ht_p.tile([84, SP], BF16, tag="hT", name="hT")
            if i < 2:   # halo cols persist in the tag's 2 rotating buffers
                nc.vector.memset(hT[:, 0:1], 0.0)
                nc.vector.memset(hT[:, SP - 1:SP], 0.0)
            nc.sync.dma_start(hT[:, 1:1 + S], ht_in[b, a])
            hTs[i] = hT

        def stage_dw_half(i, g):
            b, a = divmod(i, A)
            if g == 0:
                hdc = hdc_p.tile([124, S], BF16, tag="hdc", name="hdc")
                hdcs[i] = hdc
            hdc = hdcs[i]
            hT = hTs[i]
            s0, sn = SCH[g]
            dq = pdw.tile([96, 384], F32, tag="dq", name="dq")
            for ds in range(3):
                nc.tensor.matmul(dq[:, :sn], b1[:, ds, :],
                                 hT[:, ds + s0: ds + s0 + sn],
                                 start=(ds == 0), stop=(ds == 2))
            dst = hdc[0:96, s0:s0 + sn]
            if g == 0:
                nc.vector.tensor_copy(dst, dq[:, :sn])
            else:
                nc.scalar.copy(dst, dq[:, :sn])
                del hTs[i]
                if a < 2:   # pe rows persist in the tag's 2 rotating buffers
                    nc.gpsimd.tensor_copy(hdc[96:124, :], peds[b][:, :])

        def stage_pw_half(i, half):
            hdc = hdcs[i]
            if half == 0:
                xss[i] = []
            xts = xss[i]
            for uc in (half * 2, half * 2 + 1):
                m = UCH[uc] * T
                c0 = UOF[uc] * T
                xt = x_p.tile([126, SP], BF16, tag=f"x{uc}", name="xt")
                xts.append(xt)
                if i < 2:
                    nc.vector.memset(xt[0:m, 0:1], 0.0)
                    nc.vector.memset(xt[0:m, SP - 1:SP], 0.0)
                for g, (s0, sn) in enumerate(SCH):
                    xq = px.tile([126, 384], F32, tag="xq", name="xq")
                    nc.tensor.matmul(xq[0:m, :sn], w1[:, c0:c0 + m],
                                     hdc[:, s0:s0 + sn], start=True, stop=True)
                    dst = xt[0:m, 1 + s0:1 + s0 + sn]
                    if (uc + g) % 2 == 0:
                        nc.scalar.activation(dst, xq[0:m, :sn], RELU,
                                             bias=bv[0:m, uc:uc + 1])
                    else:
                        nc.vector.scalar_tensor_tensor(
                            dst, xq[0:m, :sn], bv[0:m, uc:uc + 1],
                            zeros[0:m, :sn], ADD, MAX)
            if half == 1:
                del hdcs[i]

        def stage_conv2(i):
            b, a = divmod(i, A)
            xts = xss.pop(i)
            zq = pz.tile([96, 770], F32, tag="zq", name="zq")
            for s0, sn in ZCH:
                for uc in range(4):
                    m = UCH[uc] * T
                    nc.tensor.matmul(zq[:, s0:s0 + sn],
                                     b2t[0:m, uc * 96:(uc + 1) * 96],
                                     xts[uc][0:m, s0:s0 + sn],
                                     start=(uc == 0), stop=(uc == 3))
            s1 = t1_p.tile([28, 768], F32, tag="s1", name="s1")
            nc.scalar.activation(s1[:], zq[0:28, 0:768], IDENT, bias=b2v[:, 0:1])
            t1 = t1_p.tile([28, 768], F32, tag="t1", name="t1")
            nc.vector.tensor_tensor(t1[:], s1[:], zq[32:60, 1:769], ADD)
            yt = ysb_p.tile([28, S], F32, tag="yt", name="yt")
            nc.vector.tensor_tensor(yt[:, :], t1[:], zq[64:92, 2:770], ADD)
            nc.sync.dma_start(y_t[b, a], yt[:])

        stage_pedma(0)
        stage_pecompute(0)
        for i in range(NIMG + 4):
            if i < NIMG:
                b, a = divmod(i, A)
                if a == 1 and b + 1 < BPC:
                    stage_pedma(b + 1)
                if a == 4 and b + 1 < BPC:
                    stage_pecompute(b + 1)
                stage_ingest(i)
            # interleave half-stages so PSUM buffer-recycle waits get slack
            if 0 <= i - 2 < NIMG:
                stage_dw_half(i - 2, 0)
            if 0 <= i - 3 < NIMG:
                stage_pw_half(i - 3, 0)
            if 0 <= i - 2 < NIMG:
                stage_dw_half(i - 2, 1)
            if 0 <= i - 3 < NIMG:
                stage_pw_half(i - 3, 1)
            if 0 <= i - 4 < NIMG:
                stage_conv2(i - 4)
    nc.compile()
    return nc


_CACHED_NC = None


def get_nc():
    global _CACHED_NC
    if _CACHED_NC is None:
        _CACHED_NC = _trace_kernel(
            bacc.Bacc("TRN2", target_bir_lowering=False, debug=False))
    return _CACHED_NC


def make_in_maps(inputs):
    consts = build_consts(
        inputs["w_hh"], inputs["b_hh"], inputs["w_vh"], inputs["b_vh"],
        inputs["w_hp"], inputs["b_hp"], inputs["w_vp"], inputs["b_vp"],
        inputs["w_oh"], inputs["b_oh"], inputs["w_ov"], inputs["b_ov"])
    # host-side layout: channel-major rows (c, t), bf16
    h = np.concatenate([np.asarray(inputs["h2"], np.float32),
                        np.asarray(inputs["h1"], np.float32)], axis=-1)
    h_t = np.ascontiguousarray(
        h.transpose(0, 3, 4, 2, 1).reshape(B, A, 6 * T, S).astype(NPBF16))
    pe_t = np.ascontiguousarray(
        np.asarray(inputs["pe"], np.float32)
        .transpose(0, 3, 2, 1).reshape(B, PEK0 * T, S).astype(NPBF16))
    in_maps = []
    for i in range(NCORES):
        sl = slice(i * BPC, (i + 1) * BPC)
        m = {
            "ht_in": np.ascontiguousarray(h_t[sl]),
            "pet_in": np.ascontiguousarray(pe_t[sl]),
        }
        m.update(consts)
        in_maps.append(m)
    return in_maps


def kernel(**inputs):
    nc = get_nc()
    in_maps = make_in_maps(inputs)
    res = run_bass_kernel_spmd(nc, in_maps, list(range(NCORES)))
    # y_t: [BPC, A, 28(k*T+t), S] -> [BPC, S, T, A, K1]
    out = np.concatenate([r["y_t"] for r in res.results], axis=0)
    out = out.reshape(B, A, K1, T, S).transpose(0, 4, 3, 1, 2)
    return np.ascontiguousarray(out)


# revision 37
# speedup vs baseline: 1.2983x; 1.0084x over previous
"""CGNN message-passing kernel for 8 trn2 NeuronCores.

Algorithm (per image (b,a), image = [S=768, T=14] grid):
  x = pw_vh(dw_hh(concat(h2,h1))) + pw_vp(dw_hp(pe)) + beta   (conv1 + pe branch)
  x = relu(x)
  y = pw_ov(dw_oh(x)) + beta2                                 (conv2)

Layout strategy: channel-major SBUF tiles [(chan,t) partitions, s free].
The DRAM<->channel-major layout conversion is done HOST-SIDE in numpy inside
kernel() (pure transpose + bf16 cast, outside HW exec), so images DMA
straight into [84, S] tiles and results DMA straight out of [28, S] tiles —
no PE transposes, no ingest/egress shuffle copies, and half the input DMA
bytes (bf16).

Depthwise 3x3 convs become accumulating matmuls with host-precomputed banded
lhsT matrices encoding the t-direction taps (T=14 blocks on the partition
axis). On-chip compute runs in bf16 (PSUM accumulation stays fp32): bf16
matmuls stream 1 cycle/row vs fp32's 4.

conv1: dw = 3 accumulating matmuls (one per s-shift ds) with out zero-padded
to 96 rows; pw streams a combined [124, s] rhs = [dw-output (96 rows, 84
live); pe-branch dw output (28 rows at base 96)] so the pe contribution rides
in the same matmul. conv1's bias is applied in the relu stage (activation
bias / scalar_tensor_tensor). conv2: dw+pw fused into banded lhsT [m, 96]
whose out dim packs the 3 s-taps in 32-row blocks; tap alignment is resolved
after the matmul by two shifted adds over a full-row [96, 770] PSUM tile.
All engine reads/writes keep quadrant-aligned (32x) partition bases.

The per-image work is emitted as a 4-stage software pipeline (ingest-DMA,
dw, pw, conv2+out-DMA), each stage one image behind the previous, so every
PE group's dependencies are a full iteration old: the PE runs gap-free at
max p-state.

Sharding: data-parallel over batch B=16 -> 2 batches per core.
"""

import numpy as np
import ml_dtypes
from contextlib import ExitStack

import concourse.bass as bass
import concourse.bacc as bacc
import concourse.tile as tile
from concourse import mybir
from concourse.bass_utils import run_bass_kernel_spmd

F32 = mybir.dt.float32
BF16 = mybir.dt.bfloat16
NPBF16 = ml_dtypes.bfloat16
B, S, T, A = 16, 768, 14, 16
HK0, PEK0, U, K1 = 6, 2, 32, 2
NCORES = 8
BPC = B // NCORES          # batches per core
NIMG = BPC * A             # images (antenna-planes) per core
SP = S + 2                 # s-padded width (zero col at 0 and S+1)
UCH = [9, 9, 9, 5]         # u-chunk sizes (32 = 9+9+9+5)
UOF = [0, 9, 18, 27]
SCH = [(0, 384), (384, 384)]   # s chunks (PSUM bank = 512 fp32 max)
ZCH = [(0, 512), (512, 258)]   # conv2 chunks inside one [96, 770] psum tile


def _tband(w_t, n_t=T):
    """[n_t, n_t] band matrix M[t, t'] = w_t[t - t' + 1] (3-tap, SAME pad)."""
    m = np.zeros((n_t, n_t), np.float32)
    for t in range(n_t):
        for tp in range(n_t):
            dt = t - tp + 1
            if 0 <= dt <= 2:
                m[t, tp] = w_t[dt]
    return m


def build_consts(w_hh, b_hh, w_vh, b_vh, w_hp, b_hp, w_vp, b_vp,
                 w_oh, b_oh, w_ov, b_ov):
    """Host-side precompute of all lhsT matrices. Returns dict name->array."""
    w_hh = w_hh[:, :, 0, :]   # [3,3,6]
    w_hp = w_hp[:, :, 0, :]   # [3,3,2]
    w_oh = w_oh[:, :, 0, :]   # [3,3,32]

    # conv1 depthwise band: [3, 84, 96] (12 zero out-cols pad to 96 rows of
    # PSUM so the dq->hdc copy is one 96-row base-0 write)
    B1 = np.zeros((3, 6 * T, 96), np.float32)
    for ds in range(3):
        for g in range(6):
            B1[ds, g * T:(g + 1) * T, g * T:(g + 1) * T] = _tband(w_hh[ds, :, g])

    # conv1 pointwise, combined contraction rows:
    #   rows 0..83: (g,t) h-dw output, 84..95: zero pad, 96..123: (c,t) pe-dw
    W1 = np.zeros((124, sum(u * T for u in UCH)), np.float32)
    col = 0
    for uc in range(4):
        for ul in range(UCH[uc]):
            u = UOF[uc] + ul
            for g in range(6):
                W1[g * T:(g + 1) * T, col:col + T] = np.eye(T, dtype=np.float32) * w_vh[g, u]
            for c in range(2):
                W1[96 + c * T:96 + (c + 1) * T, col:col + T] = \
                    np.eye(T, dtype=np.float32) * w_vp[c, u]
            col += T

    # conv1 total bias, applied at the relu stage: [126, 4], col uc, row (ul,t)
    beta = (b_vh + w_vh.T @ b_hh + b_vp + w_vp.T @ b_hp).astype(np.float32)  # [32]
    BV = np.zeros((126, 4), np.float32)
    for uc in range(4):
        for ul in range(UCH[uc]):
            BV[ul * T:(ul + 1) * T, uc] = beta[UOF[uc] + ul]

    # pe depthwise band: [3, 28, 28]
    Bpe = np.zeros((3, 2 * T, 2 * T), np.float32)
    for ds in range(3):
        for c in range(2):
            Bpe[ds, c * T:(c + 1) * T, c * T:(c + 1) * T] = _tband(w_hp[ds, :, c])

    # conv2 fused band (dw_oh folded with pw_ov), s-taps packed in out dim as
    # 32-row blocks: col block uc: [uch*14, 96]; rows (u_local,t),
    # cols (ds*32 + k*14 + t'), rows ds*32+28..ds*32+31 zero
    B2 = np.zeros((9 * T, 4 * 96), np.float32)
    for uc in range(4):
        for ul in range(UCH[uc]):
            u = UOF[uc] + ul
            for ds in range(3):
                band = _tband(w_oh[ds, :, u])          # [T, T]
                for k in range(K1):
                    r0, c0 = ul * T, uc * 96 + ds * 32 + k * T
                    B2[r0:r0 + T, c0:c0 + T] = band * w_ov[u, k]

    beta2 = (b_ov + w_ov.T @ b_oh).astype(np.float32)  # [2]
    b2vec = np.repeat(beta2, T).reshape(K1 * T, 1).astype(np.float32)  # rows (k,t)

    bf = lambda x: np.ascontiguousarray(x.astype(NPBF16))
    return {
        "wB1": bf(B1), "wW1": bf(W1), "wBpe": bf(Bpe), "wB2": bf(B2),
        "wb2": np.ascontiguousarray(b2vec),
        "wbv": np.ascontiguousarray(BV),
    }


def _trace_kernel(nc):
    ht_in = nc.dram_tensor("ht_in", [BPC, A, 84, S], BF16, kind="ExternalInput").ap()
    pet_in = nc.dram_tensor("pet_in", [BPC, 28, S], BF16, kind="ExternalInput").ap()
    wB1 = nc.dram_tensor("wB1", [3, 84, 96], BF16, kind="ExternalInput").ap()
    wW1 = nc.dram_tensor("wW1", [124, 448], BF16, kind="ExternalInput").ap()
    wBpe = nc.dram_tensor("wBpe", [3, 28, 28], BF16, kind="ExternalInput").ap()
    wB2 = nc.dram_tensor("wB2", [126, 384], BF16, kind="ExternalInput").ap()
    wb2 = nc.dram_tensor("wb2", [28, 1], F32, kind="ExternalInput").ap()
    wbv = nc.dram_tensor("wbv", [126, 4], F32, kind="ExternalInput").ap()
    y_t = nc.dram_tensor("y_t", [BPC, A, 28, S], F32, kind="ExternalOutput").ap()

    RELU = mybir.ActivationFunctionType.Relu
    IDENT = mybir.ActivationFunctionType.Identity
    ADD = mybir.AluOpType.add
    MAX = mybir.AluOpType.max

    with tile.TileContext(nc) as tc, ExitStack() as ctx:
        wp = ctx.enter_context(tc.tile_pool(name="w", bufs=1))
        ht_p = ctx.enter_context(tc.tile_pool(name="ht", bufs=3))
        hdc_p = ctx.enter_context(tc.tile_pool(name="hdc", bufs=2))
        pedw_p = ctx.enter_context(tc.tile_pool(name="pedw", bufs=2))
        x_p = ctx.enter_context(tc.tile_pool(name="xs", bufs=2))
        ysb_p = ctx.enter_context(tc.tile_pool(name="ysb", bufs=2))
        t1_p = ctx.enter_context(tc.tile_pool(name="t1", bufs=2))
        pdw = ctx.enter_context(tc.tile_pool(name="pdw", bufs=1, space="PSUM"))
        px = ctx.enter_context(tc.tile_pool(name="px", bufs=3, space="PSUM"))
        pz = ctx.enter_context(tc.tile_pool(name="pz", bufs=2, space="PSUM"))

        b1 = wp.tile([84, 3, 96], BF16)
        nc.sync.dma_start(b1[:], wB1.rearrange("d k m -> k d m"))
        w1 = wp.tile([124, 448], BF16)
        nc.sync.dma_start(w1[:], wW1)
        bpe = wp.tile([28, 3, 28], BF16)
        nc.sync.dma_start(bpe[:], wBpe.rearrange("d k m -> k d m"))
        b2t = wp.tile([126, 384], BF16)
        nc.sync.dma_start(b2t[:], wB2)
        b2v = wp.tile([28, 1], F32)
        nc.sync.dma_start(b2v[:], wb2)
        bv = wp.tile([126, 4], F32)
        nc.sync.dma_start(bv[:], wbv)
        zeros = wp.tile([126, 384], BF16)
        nc.vector.memset(zeros[:], 0.0)

        peds, peTs = {}, {}
        hTs, hdcs, xss = {}, {}, {}

        def stage_pedma(b):
            peT = ht_p.tile([28, SP], BF16, tag="peT", name="peT")
            nc.vector.memset(peT[:, 0:1], 0.0)
            nc.vector.memset(peT[:, SP - 1:SP], 0.0)
            nc.sync.dma_start(peT[:, 1:1 + S], pet_in[b])
            peTs[b] = peT

        def stage_pecompute(b):
            peT = peTs.pop(b)
            pedw = pedw_p.tile([28, S], BF16, tag="pedw", name="pedw")
            for g, (s0, sn) in enumerate(SCH):
                dq = pdw.tile([96, 384], F32, tag="dq", name="dq")
                for ds in range(3):
                    nc.tensor.matmul(dq[0:28, :sn], bpe[:, ds, :],
                                     peT[:, ds + s0: ds + s0 + sn],
                                     start=(ds == 0), stop=(ds == 2))
                nc.scalar.copy(pedw[:, s0:s0 + sn], dq[0:28, :sn])
            peds[b] = pedw

        def stage_ingest(i):
            b, a = divmod(i, A)
            hT = ht_p.tile([84, SP], BF16, tag="hT", name="hT")
            if i < 3:   # halo cols persist in the tag's 3 rotating buffers
                nc.vector.memset(hT[:, 0:1], 0.0)
                nc.vector.memset(hT[:, SP - 1:SP], 0.0)
            nc.sync.dma_start(hT[:, 1:1 + S], ht_in[b, a])
            hTs[i] = hT

        def stage_dw_half(i, g):
            b, a = divmod(i, A)
            if g == 0:
                hdc = hdc_p.tile([124, S], BF16, tag="hdc", name="hdc")
                hdcs[i] = hdc
            hdc = hdcs[i]
            hT = hTs[i]
            s0, sn = SCH[g]
            dq = pdw.tile([96, 384], F32, tag="dq", name="dq")
            for ds in range(3):
                nc.tensor.matmul(dq[:, :sn], b1[:, ds, :],
                                 hT[:, ds + s0: ds + s0 + sn],
                                 start=(ds == 0), stop=(ds == 2))
            dst = hdc[0:96, s0:s0 + sn]
            if g == 0:
                nc.vector.tensor_copy(dst, dq[:, :sn])
            else:
                nc.scalar.copy(dst, dq[:, :sn])
                del hTs[i]
                if a < 2:   # pe rows persist in the tag's 2 rotating buffers
                    nc.gpsimd.tensor_copy(hdc[96:124, :], peds[b][:, :])

        def stage_pw_half(i, half):
            hdc = hdcs[i]
            if half == 0:
                xss[i] = []
            xts = xss[i]
            for uc in (half * 2, half * 2 + 1):
                m = UCH[uc] * T
                c0 = UOF[uc] * T
                xt = x_p.tile([126, SP], BF16, tag=f"x{uc}", name="xt")
                xts.append(xt)
                if i < 2:
                    nc.vector.memset(xt[0:m, 0:1], 0.0)
                    nc.vector.memset(xt[0:m, SP - 1:SP], 0.0)
                for g, (s0, sn) in enumerate(SCH):
                    xq = px.tile([126, 384], F32, tag="xq", name="xq")
                    nc.tensor.matmul(xq[0:m, :sn], w1[:, c0:c0 + m],
                                     hdc[:, s0:s0 + sn], start=True, stop=True)
                    dst = xt[0:m, 1 + s0:1 + s0 + sn]
                    if (uc + g) % 2 == 0:
                        nc.scalar.activation(dst, xq[0:m, :sn], RELU,
                                             bias=bv[0:m, uc:uc + 1])
                    else:
                        nc.vector.scalar_tensor_tensor(
                            dst, xq[0:m, :sn], bv[0:m, uc:uc + 1],
                            zeros[0:m, :sn], ADD, MAX)
            if half == 1:
                del hdcs[i]

        def stage_conv2(i):
            b, a = divmod(i, A)
            xts = xss.pop(i)
            zq = pz.tile([96, 770], F32, tag="zq", name="zq")
            for s0, sn in ZCH:
                for uc in range(4):
                    m = UCH[uc] * T
                    nc.tensor.matmul(zq[:, s0:s0 + sn],
                                     b2t[0:m, uc * 96:(uc + 1) * 96],
                                     xts[uc][0:m, s0:s0 + sn],
                                     start=(uc == 0), stop=(uc == 3))
            s1 = t1_p.tile([28, 768], F32, tag="s1", name="s1")
            nc.scalar.activation(s1[:], zq[0:28, 0:768], IDENT, bias=b2v[:, 0:1])
            t1 = t1_p.tile([28, 768], F32, tag="t1", name="t1")
            nc.vector.tensor_tensor(t1[:], s1[:], zq[32:60, 1:769], ADD)
            yt = ysb_p.tile([28, S], F32, tag="yt", name="yt")
            nc.vector.tensor_tensor(yt[:, :], t1[:], zq[64:92, 2:770], ADD)
            nc.sync.dma_start(y_t[b, a], yt[:])

        stage_pedma(0)
        stage_pecompute(0)
        for i in range(NIMG + 4):
            if i < NIMG:
                b, a = divmod(i, A)
                if a == 1 and b + 1 < BPC:
                    stage_pedma(b + 1)
                if a == 4 and b + 1 < BPC:
                    stage_pecompute(b + 1)
                stage_ingest(i)
            # interleave half-stages so PSUM buffer-recycle waits get slack
            if 0 <= i - 2 < NIMG:
                stage_dw_half(i - 2, 0)
            if 0 <= i - 3 < NIMG:
                stage_pw_half(i - 3, 0)
                stage_pw_half(i - 3, 1)
            if 0 <= i - 2 < NIMG:
                stage_dw_half(i - 2, 1)
            if 0 <= i - 4 < NIMG:
                stage_conv2(i - 4)
    nc.compile()
    return nc


_CACHED_NC = None


def get_nc():
    global _CACHED_NC
    if _CACHED_NC is None:
        _CACHED_NC = _trace_kernel(
            bacc.Bacc("TRN2", target_bir_lowering=False, debug=False))
    return _CACHED_NC


def make_in_maps(inputs):
    consts = build_consts(
        inputs["w_hh"], inputs["b_hh"], inputs["w_vh"], inputs["b_vh"],
        inputs["w_hp"], inputs["b_hp"], inputs["w_vp"], inputs["b_vp"],
        inputs["w_oh"], inputs["b_oh"], inputs["w_ov"], inputs["b_ov"])
    # host-side layout: channel-major rows (c, t), bf16
    h = np.concatenate([np.asarray(inputs["h2"], np.float32),
                        np.asarray(inputs["h1"], np.float32)], axis=-1)
    h_t = np.ascontiguousarray(
        h.transpose(0, 3, 4, 2, 1).reshape(B, A, 6 * T, S).astype(NPBF16))
    pe_t = np.ascontiguousarray(
        np.asarray(inputs["pe"], np.float32)
        .transpose(0, 3, 2, 1).reshape(B, PEK0 * T, S).astype(NPBF16))
    in_maps = []
    for i in range(NCORES):
        sl = slice(i * BPC, (i + 1) * BPC)
        m = {
            "ht_in": np.ascontiguousarray(h_t[sl]),
            "pet_in": np.ascontiguousarray(pe_t[sl]),
        }
        m.update(consts)
        in_maps.append(m)
    return in_maps


def kernel(**inputs):
    nc = get_nc()
    in_maps = make_in_maps(inputs)
    res = run_bass_kernel_spmd(nc, in_maps, list(range(NCORES)))
    # y_t: [BPC, A, 28(k*T+t), S] -> [BPC, S, T, A, K1]
    out = np.concatenate([r["y_t"] for r in res.results], axis=0)
    out = out.reshape(B, A, K1, T, S).transpose(0, 4, 3, 1, 2)
    return np.ascontiguousarray(out)
